# revision 1
# baseline (speedup 1.0000x reference)
"""Trainium2 Bass kernel for nn_Attn (additive/Bahdanau-style attention).

Math (per batch b):
    Wh, We   = W[:, :D], W[:, D:]                       # [D,D] each
    energy   = tanh(enc @ We.T + hidden @ Wh.T + b)     # [S, D]
    scores   = energy @ v, masked to length, softmax    # [S]
    context  = scores @ enc                             # [D]

Sharding: data-parallel over batch B=16 across 8 cores (2 batches/core);
W, b, v replicated.

Device-side layout choices (prepared host-side, pure relayout of inputs):
  - encT  [BL, D, S]: enc transposed, so the contraction dim d lands on SBUF
    partitions for the pass-1 matmuls (PE contracts along partitions).
  - enc   [BL, S, D]: natural layout for the pass-2 (context) matmuls.
  - wt    [2D, D] = W.T: rows 0:D = Wh^T [d,e], rows D:2D = We^T [d,e].
  - hidT  [D, BL], bcol/vcol [128, D/128]: chunk-column layouts.

All heavy matmuls run as float32r (full-rate fp32 mode on the PE for
moving-dim >= 256).  Pass-1 computes energy^T tiles [e=128, s=512] so the
per-batch bias (hid_proj + b) is a per-partition scalar for the ACT tanh;
the v-dot accumulates on the DVE (scalar_tensor_tensor chain) with a final
128->1 partition-reduce matmul per 128-wide score chunk.  Scores live in
[128, S/128] layout throughout, so the masked softmax is a handful of
128-lane ops — exp uses the static bound M = sum|v| >= max(score) instead
of a max-reduce (softmax is shift-invariant; |tanh| <= 1 bounds scores) —
and the pass-2 stationary operand (attn column) needs no transpose.
Normalization by 1/sum is folded into the output scale.
"""

import numpy as np

B, S, D = 16, 2048, 1024
NCORES = 8
BL = B // NCORES

_NC_CACHE = {}


def _build_program(bl, s, d, st, stage="all"):
    import concourse.bacc as bacc
    import concourse.bass as bass
    import concourse.mybir as mybir
    import concourse.tile as tile

    f32 = mybir.dt.float32
    f32r = mybir.dt.float32r
    i32 = mybir.dt.int32
    Tanh = mybir.ActivationFunctionType.Tanh
    Exp = mybir.ActivationFunctionType.Exp
    Alu = mybir.AluOpType

    dc = d // 128      # contraction chunks
    ns = s // st       # pass-1 s-tiles
    sc2 = s // 128     # pass-2 s-chunks
    NEG_BIG = -1.0e30

    nc = bacc.Bacc()
    scratch_d = nc.dram_tensor("attn_scratch", [bl, s], f32)
    scratch2_d = nc.dram_tensor("hp_scratch", [bl, d], f32)
    encT_d = nc.declare_dram_parameter("encT", [bl, d, s], f32, isOutput=False)
    enc_d = nc.declare_dram_parameter("enc", [bl, s, d], f32, isOutput=False)
    wt_d = nc.declare_dram_parameter("wt", [2 * d, d], f32, isOutput=False)
    hidT_d = nc.declare_dram_parameter("hidT", [d, bl], f32, isOutput=False)
    bcol_d = nc.declare_dram_parameter("bcol", [128, dc], f32, isOutput=False)
    vcol_d = nc.declare_dram_parameter("vcol", [128, dc], f32, isOutput=False)
    len_d = nc.declare_dram_parameter("len_i", [128, bl], i32, isOutput=False)
    if stage == "all":
        out_d = nc.declare_dram_parameter("ctx_out", [bl, d], f32, isOutput=True)
    else:
        out_d = nc.declare_dram_parameter("ctx_out", [bl, s], f32, isOutput=True)

    with tile.TileContext(nc) as tc:
        with (
            tc.tile_pool(name="consts", bufs=1) as consts,
            tc.tile_pool(name="etp", bufs=4) as etp,
            tc.tile_pool(name="enp", bufs=4) as enp,
            tc.tile_pool(name="p2p", bufs=16) as p2p,
            tc.tile_pool(name="sb1", bufs=1) as sb1,
            tc.tile_pool(name="psA", bufs=4, space="PSUM") as psA,
            tc.tile_pool(name="psS", bufs=2, space="PSUM") as psS,
            tc.tile_pool(name="psM", bufs=1, space="PSUM") as psM,
        ):
            # ------------- constants -------------
            # DMA emission order matters at startup (~10 MiB must stream in
            # before steady state): hidT+Wh^T first (they gate the bias that
            # the first tanh needs), then the first encT s-tile, then We^T
            # chunk-by-chunk just-in-time for the pass-1 K-loop.
            # float32r tiles: the BIR verifier requires fp32r matmul
            # operands to be produced as fp32r, so tiles feeding the PE are
            # declared f32r and the DRAM side of each DMA is bitcast.
            hidT_sb = consts.tile([128, dc, bl], f32r)
            nc.sync.dma_start(
                out=hidT_sb,
                in_=hidT_d.rearrange("(c p) b -> p c b", p=128).bitcast(f32r),
            )
            bcol_sb = consts.tile([128, dc], f32)
            nc.sync.dma_start(out=bcol_sb, in_=bcol_d[:, :])
            # Wh^T chunks overlay the pass-2 pool: used only for hid_proj at
            # the start, then the slots recycle into en2 tiles.
            whT_tiles = []
            for c in range(dc):
                wh = p2p.tile([128, d], f32r, tag="en2", name=f"whT{c}")
                nc.sync.dma_start(
                    out=wh, in_=wt_d[c * 128:(c + 1) * 128, :].bitcast(f32r)
                )
                whT_tiles.append(wh)
            # First encT s-tile, prefetched ahead of the We^T stream.
            pre_et = {}
            et0 = etp.tile([128, dc, st], f32r, tag="et", name="et_pre")
            nc.sync.dma_start(
                out=et0,
                in_=encT_d[0, :, 0:st].rearrange("(c p) x -> p c x", p=128)
                .bitcast(f32r),
            )
            pre_et[(0, 0)] = et0
            wt_sb = consts.tile([128, dc, d], f32r)   # We^T chunks
            for c in range(dc):
                nc.sync.dma_start(
                    out=wt_sb[:, c, :],
                    in_=wt_d[(dc + c) * 128:(dc + c + 1) * 128, :].bitcast(f32r),
                )
            if ns > 1:
                et1 = etp.tile([128, dc, st], f32r, tag="et", name="et_pre1")
                nc.sync.dma_start(
                    out=et1,
                    in_=encT_d[0, :, st:2 * st].rearrange("(c p) x -> p c x", p=128)
                    .bitcast(f32r),
                )
                pre_et[(0, 1)] = et1
            if ns > 2:
                et2 = etp.tile([128, dc, st], f32r, tag="et", name="et_pre2")
                nc.sync.dma_start(
                    out=et2,
                    in_=encT_d[0, :, 2 * st:3 * st].rearrange(
                        "(c p) x -> p c x", p=128
                    ).bitcast(f32r),
                )
                pre_et[(0, 2)] = et2
            vcol_sb = consts.tile([128, dc], f32)
            nc.sync.dma_start(out=vcol_sb, in_=vcol_d[:, :])
            len_i_sb = consts.tile([128, bl], i32)
            nc.sync.dma_start(out=len_i_sb, in_=len_d[:, :])
            len_f_sb = consts.tile([128, bl], f32)
            nc.vector.tensor_copy(len_f_sb, len_i_sb)
            # Everything score-related lives in [128(p), sc2(f)] layout with
            # s = f*128 + p, so softmax ops use all 128 lanes and the
            # pass-2 stationary operand needs no transpose.
            iotaT_i = consts.tile([128, sc2], i32)
            nc.gpsimd.iota(
                iotaT_i, pattern=[[128, sc2]], base=0, channel_multiplier=1
            )
            iotaT_f = consts.tile([128, sc2], f32)
            nc.vector.tensor_copy(iotaT_f, iotaT_i)
            ones_sb = consts.tile([128, 1], f32)
            nc.vector.memset(ones_sb, 1.0)
            ones_row = consts.tile([1, 128], f32)
            nc.vector.memset(ones_row, 1.0)
            # Upper bound M = sum|v| >= any score (|tanh|<=1), used instead
            # of the true max in softmax -- removes the serial max-reduce.
            vabs = consts.tile([128, 1], f32)
            nc.vector.reduce_sum(
                out=vabs, in_=vcol_sb, axis=mybir.AxisListType.X,
                apply_absolute_value=True,
            )
            psv = psS.tile([1, st], f32, tag="s", name="psv")
            nc.tensor.matmul(psv[:, 0:1], ones_sb[:, 0:1], vabs, start=True, stop=True)
            mtot = consts.tile([1, 1], f32)
            nc.vector.tensor_copy(mtot, psv[:, 0:1])
            # broadcast -M to all 128 partitions via a K=1 matmul
            psb = psS.tile([128, 1], f32, tag="s", name="psb")
            nc.tensor.matmul(psb, ones_row[:, :], mtot[:, :], start=True, stop=True)
            negM_bc = consts.tile([128, 1], f32)
            nc.scalar.mul(negM_bc, psb, -1.0)
            validT = []
            for b_ in range(bl):
                vv = consts.tile([128, sc2], f32, name=f"validT{b_}")
                nc.vector.tensor_scalar(
                    vv, iotaT_f, len_f_sb[:, b_:b_ + 1], None, op0=Alu.is_lt
                )
                validT.append(vv)

            # ------------- hid_proj + b  ->  bias_all[e_chunk][:, b] -------------
            # hidT-stationary (tiny weight loads), kc-outer so each matmul
            # only needs Wh^T chunk kc as the DMA delivers it.  One
            # accumulation group per 512-wide PSUM bank half (start=True
            # clears has_written for the WHOLE bank, so groups must not
            # interleave within a bank).  Output is [b, e]; bounce through
            # DRAM to get the [e-partition] layout the tanh bias needs.
            nh2 = max(1, d // 512)
            hwb = d // nh2
            ps_hb = psM.tile([bl, d], f32, tag="m")

            def emit_hid_mms(kcs):
                for kc in kcs:
                    for h in range(nh2):
                        nc.tensor.matmul(
                            ps_hb[:, h * hwb:(h + 1) * hwb],
                            hidT_sb[:, kc, :],
                            whT_tiles[kc][:, h * hwb:(h + 1) * hwb],
                            start=(kc == 0),
                            stop=(kc == dc - 1),
                            skip_group_check=True,
                        )

            hid_queue = list(range(dc))
            hp_sb = consts.tile([bl, d], f32)
            Identity = mybir.ActivationFunctionType.Identity
            bias_all = consts.tile([128, dc, bl], f32)

            def emit_bias_chain():
                nc.scalar.copy(hp_sb, ps_hb)
                nc.gpsimd.dma_start(out=scratch2_d[:, :], in_=hp_sb)
                bias_raw = consts.tile([128, dc, bl], f32)
                for b_ in range(bl):
                    nc.gpsimd.dma_start(
                        out=bias_raw[:, :, b_],
                        in_=scratch2_d[b_, :].rearrange("(c p) -> p c", p=128),
                    )
                # ACT (not DVE tensor_scalar): the TensorScalar ISA struct
                # has one sync-wait slot; this op needs PE + DMA waits.
                for ec in range(dc):
                    nc.scalar.activation(
                        bias_all[:, ec, :],
                        bias_raw[:, ec, :],
                        Identity,
                        bias=bcol_sb[:, ec:ec + 1],
                    )

            nst = st // 128   # 128-wide score chunks per s-tile

            def flush_pending(pending):
                # Emit the deferred partition-reduces + copies for the
                # previous s-tile; deferring gives the DVE v-dot chain time
                # to finish without stalling the PE.  Each chunk c of acc
                # column-sums into scoresT[:, f] (s = f*128 + p).
                acc_p, sco_p, sti_p = pending
                for c_ in range(nst):
                    sps = psS.tile([128, 1], f32, tag="s")
                    nc.tensor.matmul(
                        sps,
                        acc_p[:, c_ * 128:(c_ + 1) * 128],
                        ones_sb[:, 0:1],
                        start=True,
                        stop=True,
                    )
                    nc.vector.tensor_copy(
                        sco_p[:, sti_p * nst + c_:sti_p * nst + c_ + 1], sps
                    )

            pending = None
            emit_hid_mms(list(range(dc)))
            hid_queue = []
            emit_bias_chain()
            for bb in range(bl):
                # ------------- pass 1: scores -------------
                scores_sb = sb1.tile([128, sc2], f32, tag="scores", bufs=2)
                for sti in range(ns):
                    et = pre_et.pop((bb, sti), None)
                    if et is None:
                        et = etp.tile([128, dc, st], f32r, tag="et")
                        nc.sync.dma_start(
                            out=et,
                            in_=encT_d[bb, :, sti * st:(sti + 1) * st].rearrange(
                                "(c p) x -> p c x", p=128
                            ).bitcast(f32r),
                        )
                    acc = enp.tile([128, st], f32, tag="acc")
                    for ec in range(dc):
                        ps = psA.tile([128, st], f32, tag="proj")
                        for kc in range(dc):
                            nc.tensor.matmul(
                                ps,
                                wt_sb[:, kc, ec * 128:(ec + 1) * 128],
                                et[:, kc, :],
                                start=(kc == 0),
                                stop=(kc == dc - 1),
                            )
                        if ec == min(2, dc - 1) and pending is not None:
                            flush_pending(pending)
                            pending = None
                        en = enp.tile([128, st], f32, tag="en")
                        nc.scalar.activation(
                            en, ps, Tanh, bias=bias_all[:, ec, bb:bb + 1]
                        )
                        # v-dot on DVE: acc[p, s] accumulates v[ec*128+p]*en
                        if ec == 0:
                            nc.vector.tensor_scalar_mul(
                                acc, en, vcol_sb[:, 0:1]
                            )
                        else:
                            nc.vector.scalar_tensor_tensor(
                                acc,
                                en,
                                vcol_sb[:, ec:ec + 1],
                                acc,
                                op0=Alu.mult,
                                op1=Alu.add,
                            )
                    if pending is not None:
                        flush_pending(pending)
                    pending = (acc, scores_sb, sti)
                    if bb == 0 and sti == 2 and whT_tiles:
                        # Late "reads" of the Wh^T tiles so their pool slots
                        # (shared with the pass-2 en2 tiles) release only
                        # now -- keeps the en2 prefetch DMAs from competing
                        # with the startup encT/We^T streams for HBM BW.
                        hold = consts.tile([1, 1], f32, name="hold")
                        for whx in whT_tiles:
                            nc.vector.tensor_copy(hold, whx[0:1, 0:1])
                        whT_tiles = []
                if pending is not None:
                    flush_pending(pending)
                    pending = None

                if stage == "p1":
                    nc.gpsimd.dma_start(
                        out=out_d[bb, :].rearrange("(f p) -> p f", p=128),
                        in_=scores_sb,
                    )
                    continue

                # ------------- masked softmax (normalization deferred) ---------
                # exp(score - M) with the global bound M = sum|v| (no
                # max-reduce); mask + per-partition row-sum fused in one
                # DVE pass; all ops are [128, sc2] so they cost ~100 ns.
                attn_raw = sb1.tile([128, sc2], f32, tag="araw")
                nc.scalar.activation(
                    attn_raw, scores_sb, Exp, bias=negM_bc[:, 0:1]
                )
                attn_exp = sb1.tile([128, sc2], f32, tag="aexp")
                psums = sb1.tile([128, 1], f32, tag="psums")
                nc.vector.scalar_tensor_tensor(
                    attn_exp,
                    attn_raw,
                    1.0,
                    validT[bb],
                    op0=Alu.mult,
                    op1=Alu.mult,
                    accum_out=psums,
                )
                # attnT (f32r) is just a rounding copy -- no transpose needed
                attnT = sb1.tile([128, sc2], f32r, tag="attnT")
                nc.scalar.copy(attnT, attn_exp)
                # total sum across partitions -> reciprocal
                psm = psS.tile([128, 1], f32, tag="s", name="psm")
                nc.tensor.matmul(
                    psm[0:1, 0:1], psums, ones_sb[:, 0:1], start=True, stop=True
                )
                if stage == "sm":
                    nc.gpsimd.dma_start(
                        out=out_d[bb, :].rearrange("(f p) -> p f", p=128),
                        in_=attn_exp,
                    )
                    continue
                rinv = sb1.tile([1, 1], f32, tag="rinv")
                nc.vector.reciprocal(rinv, psm[0:1, 0:1])

                # ------------- pass 2: context -------------
                nh = 2 if d > 512 else 1
                hw_ = d // nh
                cps = psM.tile([1, d], f32, tag="m", name="cps")
                for sci in range(sc2):
                    en2 = p2p.tile([128, d], f32r, tag="en2")
                    nc.sync.dma_start(
                        out=en2,
                        in_=enc_d[bb, sci * 128:(sci + 1) * 128, :].bitcast(f32r),
                    )
                    for h in range(nh):
                        nc.tensor.matmul(
                            cps[:, h * hw_:(h + 1) * hw_],
                            attnT[:, sci:sci + 1],
                            en2[:, h * hw_:(h + 1) * hw_],
                            start=(sci == 0),
                            stop=(sci == sc2 - 1),
                        )
                ctx_sb = sb1.tile([1, d], f32, tag="ctx")
                nc.scalar.mul(ctx_sb, cps, rinv[0:1, 0:1])
                nc.gpsimd.dma_start(out=out_d[bb:bb + 1, :], in_=ctx_sb)

    nc.compile()
    return nc


def _get_nc(bl=BL, s=S, d=D, st=512, stage="all"):
    key = (bl, s, d, st, stage)
    if key not in _NC_CACHE:
        _NC_CACHE[key] = _build_program(bl, s, d, st, stage)
    return _NC_CACHE[key]


def _make_in_maps(encoder_outputs, hidden, lengths, W, b, v):
    enc = np.asarray(encoder_outputs, dtype=np.float32)
    hid = np.asarray(hidden, dtype=np.float32)
    len_ = np.asarray(lengths, dtype=np.int32)
    Wn = np.asarray(W, dtype=np.float32)
    bn = np.asarray(b, dtype=np.float32)
    vn = np.asarray(v, dtype=np.float32)

    dc = D // 128
    wt = np.ascontiguousarray(Wn.T)                      # [2D, D]
    bcol = np.ascontiguousarray(bn.reshape(dc, 128).T)   # [128, dc]
    vcol = np.ascontiguousarray(vn.reshape(dc, 128).T)
    in_maps = []
    for i in range(NCORES):
        sl = slice(BL * i, BL * (i + 1))
        e = enc[sl]
        in_maps.append(
            dict(
                encT=np.ascontiguousarray(e.transpose(0, 2, 1)),
                enc=np.ascontiguousarray(e),
                wt=wt,
                hidT=np.ascontiguousarray(hid[sl].T),
                bcol=bcol,
                vcol=vcol,
                len_i=np.ascontiguousarray(
                    np.broadcast_to(len_[sl].reshape(1, BL), (128, BL)).copy()
                ),
            )
        )
    return in_maps


def run(inputs, trace=False):
    """Run on 8 NeuronCores; returns (output [B,1,D], BassKernelResults)."""
    from concourse.bass_utils import run_bass_kernel_spmd

    nc = _get_nc()
    in_maps = _make_in_maps(**inputs)
    r = run_bass_kernel_spmd(
        nc, in_maps, core_ids=list(range(NCORES)), trace=trace
    )
    out = np.concatenate(
        [np.asarray(r.results[i]["ctx_out"]) for i in range(NCORES)], axis=0
    )
    return out[:, None, :].astype(np.float32), r


def kernel(encoder_outputs, hidden, lengths, W, b, v):
    out, _ = run(
        dict(
            encoder_outputs=encoder_outputs,
            hidden=hidden,
            lengths=lengths,
            W=W,
            b=b,
            v=v,
        )
    )
    return out



# revision 3
# speedup vs baseline: 1.5407x; 1.5407x over previous
"""Trainium2 Bass kernel for nn_Attn (additive/Bahdanau-style attention).

Math (per batch b):
    Wh, We   = W[:, :D], W[:, D:]                       # [D,D] each
    energy   = tanh(enc @ We.T + hidden @ Wh.T + b)     # [S, D]
    scores   = energy @ v, masked to length, softmax    # [S]
    context  = scores @ enc                             # [D]

Sharding / packing: data-parallel over batch B=16 across 8 cores, but
length-aware.  Positions >= lengths[b] are masked out of the softmax, so
only ceil(len/512)*512 positions per batch ever matter.  The host sorts
batches by padded tile count and pairs longest-with-shortest so every
core gets the same number NT of 512-wide s-tiles (5 for the reference
lengths instead of 8 for the naive full-S split).  Each core's two
batches are packed back-to-back into one flat tile list; the batch
structure (tile ownership, per-position validity) is carried entirely by
host-prepared relayout inputs (replicated hidden columns, owner masks,
position indices), so one SPMD program serves all cores.

Device-side structure:
  - pass 1 computes energy^T tiles [e=128, s=512] with We^T-stationary
    matmuls in bf16 (full PE rate, half the DMA/SBUF of fp32), looped
    (group, ec, kc, tile) so each weight chunk loads once per group.
  - the tanh bias (hid @ Wh^T + b) is computed on-device as
    [e-partition, tile] via stationary-Wh^T matmuls (no DRAM bounce).
  - the v-dot accumulates on the DVE; a per-tile partition-reduce matmul
    yields scores in [128, flat-chunk] layout, so the masked softmax is
    a handful of 128-lane ops.  exp uses the static bound M = sum|v| >=
    max(score) (softmax shift-invariance; |tanh| <= 1) -- no max-reduce.
  - pass 2 accumulates BOTH batch contexts at once into one [2, D] PSUM
    group: the stationary operand is [s=128, 2] of masked, batch-selected
    exp weights.  Normalization by 1/sum folds into the output scale.
  - tiles are processed in groups ([0], [1,2], [3,4], ...): the first
    group starts compute after a minimal DMA prefix, later groups reuse
    each weight load across member tiles, and every group's softmax +
    pass-2 work is emitted interleaved into the next group's matmul
    stream so the PE never waits on the DVE chain.
"""

import numpy as np

B, S, D = 16, 2048, 1024
NCORES = 8
BL = B // NCORES   # batches per core
ST = 512           # s-tile width (pass-1 moving dim; one PSUM bank)
DC = D // 128      # contraction / e chunks
NPT = ST // 128    # 128-wide flat chunks per s-tile

_NC_CACHE = {}


def _build_program(nt, stage="all"):
    import concourse.bacc as bacc
    import concourse.bass as bass
    import concourse.mybir as mybir
    import concourse.tile as tile

    f32 = mybir.dt.float32
    bf16 = mybir.dt.bfloat16
    Tanh = mybir.ActivationFunctionType.Tanh
    Exp = mybir.ActivationFunctionType.Exp
    Identity = mybir.ActivationFunctionType.Identity
    Alu = mybir.AluOpType

    nf = nt * NPT        # flat 128-wide chunks per core
    d = D

    # tile groups: [0] alone (fast start after a small DMA prefix), then
    # pairs; a trailing singleton if nt is even.
    groups = [[0]]
    t = 1
    while t < nt:
        g = list(range(t, min(t + 2, nt)))
        groups.append(g)
        t += len(g)

    nc = bacc.Bacc()
    encTf_d = nc.declare_dram_parameter("encTf", [nt, DC, 128, ST], bf16, isOutput=False)
    encf_d = nc.declare_dram_parameter("encf", [nf, 128, d], bf16, isOutput=False)
    weTs_d = nc.declare_dram_parameter("weTs", [DC, 128, d], bf16, isOutput=False)
    whTs_d = nc.declare_dram_parameter("whTs", [DC, 128, d], bf16, isOutput=False)
    hidf_d = nc.declare_dram_parameter("hidf", [DC, 128, nt], bf16, isOutput=False)
    bcol_d = nc.declare_dram_parameter("bcol", [128, DC], f32, isOutput=False)
    vcol_d = nc.declare_dram_parameter("vcol", [128, DC], f32, isOutput=False)
    posf_d = nc.declare_dram_parameter("posf", [128, nf], f32, isOutput=False)
    lenmap_d = nc.declare_dram_parameter("lenmap", [128, nf], f32, isOutput=False)
    own0_d = nc.declare_dram_parameter("own0", [128, nf], f32, isOutput=False)
    if stage == "all":
        out_d = nc.declare_dram_parameter("ctx_out", [BL, d], f32, isOutput=True)
    else:
        out_d = nc.declare_dram_parameter("ctx_out", [128, nf], f32, isOutput=True)

    with tile.TileContext(nc) as tc:
        with (
            tc.tile_pool(name="consts", bufs=1) as consts,
            tc.tile_pool(name="etp", bufs=nt) as etp,
            tc.tile_pool(name="enf", bufs=nf) as enf,
            tc.tile_pool(name="enp", bufs=4) as enp,
            tc.tile_pool(name="psA", bufs=4, space="PSUM") as psA,
            tc.tile_pool(name="psS", bufs=2, space="PSUM") as psS,
            tc.tile_pool(name="psM", bufs=1, space="PSUM") as psM,
        ):
            # ---------------- DMA emission (one queue, priority order) ----
            # whTs+hidf first: the hid-bias matmuls cover the weTs/encTf[0]
            # DMA window on the PE.
            whTs_sb = consts.tile([128, DC, d], bf16)
            nc.sync.dma_start(out=whTs_sb, in_=whTs_d.rearrange("c p e -> p c e"))
            hidf_sb = consts.tile([128, DC, nt], bf16)
            nc.sync.dma_start(out=hidf_sb, in_=hidf_d.rearrange("c p t -> p c t"))
            vcol_sb = consts.tile([128, DC], f32)
            nc.sync.dma_start(out=vcol_sb, in_=vcol_d[:, :])
            bcol_sb = consts.tile([128, DC], f32)
            nc.sync.dma_start(out=bcol_sb, in_=bcol_d[:, :])
            posf_sb = consts.tile([128, nf], f32)
            nc.sync.dma_start(out=posf_sb, in_=posf_d[:, :])
            lenmap_sb = consts.tile([128, nf], f32)
            nc.sync.dma_start(out=lenmap_sb, in_=lenmap_d[:, :])
            own0_sb = consts.tile([128, nf], f32)
            nc.sync.dma_start(out=own0_sb, in_=own0_d[:, :])
            weTs_sb = consts.tile([128, DC, d], bf16)
            nc.sync.dma_start(out=weTs_sb, in_=weTs_d.rearrange("c p e -> p c e"))
            et_tiles = []
            for t_ in range(nt):
                et = etp.tile([128, DC, ST], bf16, tag="et")
                nc.sync.dma_start(
                    out=et, in_=encTf_d[t_].rearrange("c p x -> p c x")
                )
                et_tiles.append(et)
            en2_tiles = []
            for f in range(nf):
                en2 = enf.tile([128, d], bf16, tag="en2")
                nc.sync.dma_start(out=en2, in_=encf_d[f])
                en2_tiles.append(en2)

            # ---------------- small constants ----------------------------
            ones_sb = consts.tile([128, 1], f32)
            nc.vector.memset(ones_sb, 1.0)
            ones_row = consts.tile([1, 128], f32)
            nc.vector.memset(ones_row, 1.0)
            # Upper bound M = sum|v| >= any score (|tanh|<=1): replaces the
            # serial max-reduce in the softmax.
            vabs = consts.tile([128, 1], f32)
            nc.vector.reduce_sum(
                out=vabs, in_=vcol_sb, axis=mybir.AxisListType.X,
                apply_absolute_value=True,
            )
            psv = psS.tile([1, 1], f32, tag="s", name="psv")
            nc.tensor.matmul(psv, ones_sb[:, 0:1], vabs, start=True, stop=True)
            mtot = consts.tile([1, 1], f32)
            nc.vector.tensor_copy(mtot, psv)
            psb = psS.tile([128, 1], f32, tag="s", name="psb")
            nc.tensor.matmul(psb, ones_row[:, :], mtot[:, :], start=True, stop=True)
            negM = consts.tile([128, 1], f32)
            nc.scalar.mul(negM, psb, -1.0)

            # masks from host-relayout index tensors: valid = pos < len,
            # then split by batch-slot ownership.
            valid_sb = consts.tile([128, nf], f32)
            nc.vector.scalar_tensor_tensor(
                valid_sb, posf_sb, 1.0, lenmap_sb, op0=Alu.mult, op1=Alu.is_lt
            )
            mask0 = consts.tile([128, nf], f32)
            nc.vector.scalar_tensor_tensor(
                mask0, valid_sb, 1.0, own0_sb, op0=Alu.mult, op1=Alu.mult
            )
            mask1 = consts.tile([128, nf], f32)
            nc.vector.scalar_tensor_tensor(
                mask1, valid_sb, 1.0, mask0, op0=Alu.mult, op1=Alu.subtract
            )

            # ---------------- hid bias: (hid @ Wh^T + b)^T ----------------
            # stationary Wh^T chunk [128d, 128e] x moving hidf [128d, nt]
            # -> [128e, nt] directly in the layout the tanh bias needs.
            bias_all = consts.tile([128, DC, nt], f32)
            for ec in range(DC):
                psh = psS.tile([128, nt], f32, tag="s", name=f"psh{ec}")
                for kc in range(DC):
                    nc.tensor.matmul(
                        psh,
                        whTs_sb[:, kc, ec * 128:(ec + 1) * 128],
                        hidf_sb[:, kc, :],
                        start=(kc == 0),
                        stop=(kc == DC - 1),
                    )
                nc.scalar.activation(
                    bias_all[:, ec, :], psh, Identity, bias=bcol_sb[:, ec:ec + 1]
                )

            # ---------------- pass 1 + pipelined softmax / pass 2 ---------
            scores_sb = consts.tile([128, nf], f32)
            exp_sb = consts.tile([128, nf], f32)
            attn2b = consts.tile([128, nf, 2], bf16)
            mexp0 = consts.tile([128, nf], f32)
            mexp1 = consts.tile([128, nf], f32)
            psums01 = consts.tile([128, 2], f32)
            cps = psM.tile([BL, d], f32, tag="m", name="cps")

            pend = None            # (tiles, accs) of the previous group
            p2_emitted = 0         # flat chunks whose pass-2 mm is emitted

            def emit_reduces(tiles, accs):
                # partition-reduce each acc column block into scores_sb.
                # All chunks of the pending group go into one PSUM tile
                # (separate cols) so nothing serializes on ring reuse.
                sps = psS.tile([128, NPT * len(tiles)], f32, tag="s")
                for j, t_ in enumerate(tiles):
                    for c in range(NPT):
                        nc.tensor.matmul(
                            sps[:, j * NPT + c:j * NPT + c + 1],
                            accs[t_][:, c * 128:(c + 1) * 128],
                            ones_sb[:, 0:1],
                            start=True,
                            stop=True,
                        )
                f0 = tiles[0] * NPT
                f1 = tiles[-1] * NPT + NPT
                nc.vector.tensor_copy(scores_sb[:, f0:f1], sps)

            def emit_softmax(tiles):
                f0 = tiles[0] * NPT
                f1 = tiles[-1] * NPT + NPT
                nc.scalar.activation(
                    exp_sb[:, f0:f1], scores_sb[:, f0:f1], Exp, bias=negM[:, 0:1]
                )
                nc.vector.scalar_tensor_tensor(
                    attn2b[:, f0:f1, 0], exp_sb[:, f0:f1], 1.0, mask0[:, f0:f1],
                    op0=Alu.mult, op1=Alu.mult,
                )
                nc.vector.scalar_tensor_tensor(
                    attn2b[:, f0:f1, 1], exp_sb[:, f0:f1], 1.0, mask1[:, f0:f1],
                    op0=Alu.mult, op1=Alu.mult,
                )

            def emit_pass2(tiles):
                nonlocal p2_emitted
                f0 = tiles[0] * NPT
                f1 = tiles[-1] * NPT + NPT
                for f in range(f0, f1):
                    for h in range(2):
                        nc.tensor.matmul(
                            cps[:, h * 512:(h + 1) * 512],
                            attn2b[:, f, :],
                            en2_tiles[f][:, h * 512:(h + 1) * 512],
                            start=(f == 0),
                            stop=(f == nf - 1),
                        )
                p2_emitted = f1

            for tiles in groups:
                accs = {}
                for ec in range(DC):
                    pss = {
                        t_: psA.tile([128, ST], f32, tag="proj", name=f"ps{t_}_{ec}")
                        for t_ in tiles
                    }
                    for kc in range(DC):
                        for t_ in tiles:
                            nc.tensor.matmul(
                                pss[t_],
                                weTs_sb[:, kc, ec * 128:(ec + 1) * 128],
                                et_tiles[t_][:, kc, :],
                                start=(kc == 0),
                                stop=(kc == DC - 1),
                            )
                    # deferred post-work of the previous group, staged so the
                    # PE queue always has matmul runway ahead of the deps.
                    if pend is not None:
                        if ec == 0:
                            emit_reduces(*pend)
                        elif ec == 1:
                            emit_softmax(pend[0])
                        elif ec == 2:
                            emit_pass2(pend[0])
                            pend = None
                    for t_ in tiles:
                        en = enp.tile([128, ST], f32, tag="en")
                        nc.scalar.activation(
                            en, pss[t_], Tanh, bias=bias_all[:, ec, t_:t_ + 1]
                        )
                        if ec == 0:
                            acc = enp.tile([128, ST], f32, tag="acc", bufs=5)
                            accs[t_] = acc
                            nc.vector.tensor_scalar_mul(acc, en, vcol_sb[:, 0:1])
                        else:
                            nc.vector.scalar_tensor_tensor(
                                accs[t_], en, vcol_sb[:, ec:ec + 1], accs[t_],
                                op0=Alu.mult, op1=Alu.add,
                            )
                pend = (tiles, accs)

            # tail: post-work of the last group
            emit_reduces(*pend)
            emit_softmax(pend[0])
            if stage == "p1":
                nc.gpsimd.dma_start(out=out_d[:, :], in_=scores_sb)
            elif stage == "sm":
                nc.gpsimd.dma_start(out=out_d[:, :], in_=exp_sb)
            else:
                emit_pass2(pend[0])
                assert p2_emitted == nf

                # softmax denominators for both batch slots -> [2, 1]
                nc.vector.scalar_tensor_tensor(
                    mexp0, exp_sb, 1.0, mask0, op0=Alu.mult, op1=Alu.mult,
                    accum_out=psums01[:, 0:1],
                )
                nc.vector.scalar_tensor_tensor(
                    mexp1, exp_sb, 1.0, mask1, op0=Alu.mult, op1=Alu.mult,
                    accum_out=psums01[:, 1:2],
                )
                pst = psS.tile([BL, 1], f32, tag="s", name="pst")
                nc.tensor.matmul(pst, psums01, ones_sb[:, 0:1], start=True, stop=True)
                rinv2 = consts.tile([BL, 1], f32)
                nc.vector.reciprocal(rinv2, pst)
                ctx_sb = consts.tile([BL, d], f32)
                nc.vector.tensor_scalar_mul(ctx_sb, cps, rinv2)
                nc.gpsimd.dma_start(out=out_d[:, :], in_=ctx_sb)

    nc.compile()
    return nc


def _get_nc(nt, stage="all"):
    key = (nt, stage)
    if key not in _NC_CACHE:
        _NC_CACHE[key] = _build_program(nt, stage)
    return _NC_CACHE[key]


def _plan(lengths):
    """Pair batches (longest padded length with shortest) so every core's
    two batches need the same, minimal number of 512-wide s-tiles."""
    l = np.asarray(lengths, dtype=np.int64)
    c = (np.clip(l, 1, S) + ST - 1) // ST          # tiles per batch, >= 1
    order = np.argsort(-c, kind="stable")
    pairs = [(int(order[i]), int(order[B - 1 - i])) for i in range(NCORES)]
    nt = int(max(c[a] + c[b] for a, b in pairs))
    return pairs, c, nt


def _make_in_maps(encoder_outputs, hidden, lengths, W, b, v):
    import ml_dtypes

    bf16 = ml_dtypes.bfloat16
    enc = np.asarray(encoder_outputs, dtype=np.float32)
    hid = np.asarray(hidden, dtype=np.float32)
    len_ = np.asarray(lengths, dtype=np.int64)
    Wn = np.asarray(W, dtype=np.float32)
    bn = np.asarray(b, dtype=np.float32)
    vn = np.asarray(v, dtype=np.float32)

    pairs, c, nt = _plan(len_)
    nf = nt * NPT

    weTs = np.ascontiguousarray(Wn[:, D:].T).astype(bf16).reshape(DC, 128, D)
    whTs = np.ascontiguousarray(Wn[:, :D].T).astype(bf16).reshape(DC, 128, D)
    bcol = np.ascontiguousarray(bn.reshape(DC, 128).T)
    vcol = np.ascontiguousarray(vn.reshape(DC, 128).T)

    in_maps = []
    for a, b_ in pairs:
        na, nb = int(c[a]), int(c[b_])
        packed = np.zeros((nt * ST, D), dtype=np.float32)
        packed[:na * ST] = enc[a, :na * ST]
        packed[na * ST:(na + nb) * ST] = enc[b_, :nb * ST]
        packed = packed.astype(bf16)
        encTf = np.ascontiguousarray(
            packed.reshape(nt, ST, DC, 128).transpose(0, 2, 3, 1)
        )
        encf = packed.reshape(nf, 128, D)

        hidf = np.zeros((D, nt), dtype=np.float32)
        hidf[:, :na] = hid[a][:, None]
        hidf[:, na:na + nb] = hid[b_][:, None]
        hidf = np.ascontiguousarray(hidf.astype(bf16).reshape(DC, 128, nt))

        posf = np.full((128, nf), 1.0e9, dtype=np.float32)
        lenmap = np.zeros((128, nf), dtype=np.float32)
        own0 = np.zeros((128, nf), dtype=np.float32)
        p = np.arange(128, dtype=np.float32)
        for f in range(nf):
            t = f // NPT
            if t < na:
                posf[:, f] = f * 128 + p
                lenmap[:, f] = float(len_[a])
                own0[:, f] = 1.0
            elif t < na + nb:
                posf[:, f] = (f - na * NPT) * 128 + p
                lenmap[:, f] = float(len_[b_])

        in_maps.append(
            dict(
                encTf=encTf, encf=np.ascontiguousarray(encf),
                weTs=weTs, whTs=whTs, hidf=hidf,
                bcol=bcol, vcol=vcol,
                posf=posf, lenmap=lenmap, own0=own0,
            )
        )
    return in_maps, pairs, nt


def run(inputs, trace=False, stage="all"):
    """Run on 8 NeuronCores; returns (output [B,1,D], BassKernelResults)."""
    from concourse.bass_utils import run_bass_kernel_spmd

    in_maps, pairs, nt = _make_in_maps(**inputs)
    nc = _get_nc(nt, stage)
    r = run_bass_kernel_spmd(
        nc, in_maps, core_ids=list(range(NCORES)), trace=trace
    )
    if stage != "all":
        out = np.stack(
            [np.asarray(r.results[i]["ctx_out"]) for i in range(NCORES)], axis=0
        )
        return out, r, pairs
    out = np.empty((B, 1, D), dtype=np.float32)
    for i, (a, b_) in enumerate(pairs):
        ctx = np.asarray(r.results[i]["ctx_out"])
        out[a, 0] = ctx[0]
        out[b_, 0] = ctx[1]
    return out, r


def kernel(encoder_outputs, hidden, lengths, W, b, v):
    out, _ = run(
        dict(
            encoder_outputs=encoder_outputs,
            hidden=hidden,
            lengths=lengths,
            W=W,
            b=b,
            v=v,
        )
    )
    return out


# revision 10
# speedup vs baseline: 1.6044x; 1.0413x over previous
"""Trainium2 Bass kernel for nn_Attn (additive/Bahdanau-style attention).

Math (per batch b):
    Wh, We   = W[:, :D], W[:, D:]                       # [D,D] each
    energy   = tanh(enc @ We.T + hidden @ Wh.T + b)     # [S, D]
    scores   = energy @ v, masked to length, softmax    # [S]
    context  = scores @ enc                             # [D]

Sharding / packing: data-parallel over batch B=16 across 8 cores, but
length-aware.  Positions >= lengths[b] are masked out of the softmax, so
only ceil(len/512)*512 positions per batch ever matter.  The host sorts
batches by padded tile count and pairs longest-with-shortest so every
core gets the same number NT of 512-wide s-tiles (5 for the reference
lengths instead of 8 for the naive full-S split).  Each core's two
batches are packed back-to-back into one flat tile list; the batch
structure (tile ownership, per-position validity) is carried entirely by
host-prepared relayout inputs (replicated hidden columns, owner masks,
position indices), so one SPMD program serves all cores.

Device-side structure:
  - pass 1 computes energy^T tiles [e=128, s=512] with We^T-stationary
    matmuls in bf16 (full PE rate, half the DMA/SBUF of fp32), looped
    (group, ec, kc, tile) so each weight chunk loads once per group.
  - the tanh bias (hid @ Wh^T + b) is computed on-device as
    [e-partition, tile] via stationary-Wh^T matmuls (no DRAM bounce).
  - the v-dot accumulates on the DVE; a per-tile partition-reduce matmul
    yields scores in [128, flat-chunk] layout, so the masked softmax is
    a handful of 128-lane ops.  exp uses the static bound M = sum|v| >=
    max(score) (softmax shift-invariance; |tanh| <= 1) -- no max-reduce.
  - pass 2 accumulates BOTH batch contexts at once into one [2, D] PSUM
    group: the stationary operand is [s=128, 2] of masked, batch-selected
    exp weights.  Normalization by 1/sum folds into the output scale.
  - tiles are processed in groups ([0], [1,2], [3,4], ...): the first
    group starts compute after a minimal DMA prefix, later groups reuse
    each weight load across member tiles, and every group's softmax +
    pass-2 work is emitted interleaved into the next group's matmul
    stream so the PE never waits on the DVE chain.
"""

import numpy as np

B, S, D = 16, 2048, 1024
NCORES = 8
BL = B // NCORES   # batches per core
ST = 512           # s-tile width (pass-1 moving dim; one PSUM bank)
DC = D // 128      # contraction / e chunks
NPT = ST // 128    # 128-wide flat chunks per s-tile

_NC_CACHE = {}


def _build_program(nt, stage="all"):
    import concourse.bacc as bacc
    import concourse.bass as bass
    import concourse.mybir as mybir
    import concourse.tile as tile

    f32 = mybir.dt.float32
    bf16 = mybir.dt.bfloat16
    Tanh = mybir.ActivationFunctionType.Tanh
    Exp = mybir.ActivationFunctionType.Exp
    Identity = mybir.ActivationFunctionType.Identity
    Alu = mybir.AluOpType

    nf = nt * NPT        # flat 128-wide chunks per core
    d = D

    # tile groups: [0] alone (fast start after a small DMA prefix), then
    # pairs, with a singleton LAST group so the tail dependency chain
    # (reduce -> exp -> attn2 -> pass-2) covers only one tile.
    groups = [[0]]
    rem = list(range(1, nt))
    while rem:
        k = 2 if len(rem) > 2 else 1
        groups.append(rem[:k])
        rem = rem[k:]

    nc = bacc.Bacc()
    # all big inputs are host-prearranged partition-major so every DMA is a
    # straight [128, X] copy with one contiguous line per partition.
    encTf_d = nc.declare_dram_parameter("encTf", [nt, 128, DC, ST], bf16, isOutput=False)
    encf_d = nc.declare_dram_parameter("encf", [nf, 128, d], bf16, isOutput=False)
    weTs_d = nc.declare_dram_parameter("weTs", [128, DC, d], bf16, isOutput=False)
    whTs_d = nc.declare_dram_parameter("whTs", [128, DC, d], bf16, isOutput=False)
    hidf_d = nc.declare_dram_parameter("hidf", [128, DC, nt], bf16, isOutput=False)
    bcol_d = nc.declare_dram_parameter("bcol", [128, DC], f32, isOutput=False)
    vcol_d = nc.declare_dram_parameter("vcol", [128, DC], f32, isOutput=False)
    posf_d = nc.declare_dram_parameter("posf", [128, nf], f32, isOutput=False)
    lenmap_d = nc.declare_dram_parameter("lenmap", [128, nf], f32, isOutput=False)
    own0_d = nc.declare_dram_parameter("own0", [128, nf], f32, isOutput=False)
    if stage == "all":
        out_d = nc.declare_dram_parameter("ctx_out", [BL, d], f32, isOutput=True)
    else:
        out_d = nc.declare_dram_parameter("ctx_out", [128, nf], f32, isOutput=True)

    with tile.TileContext(nc) as tc:
        with (
            tc.tile_pool(name="consts", bufs=1) as consts,
            tc.tile_pool(name="etp", bufs=nt) as etp,
            tc.tile_pool(name="enf", bufs=nf) as enf,
            tc.tile_pool(name="enp", bufs=4) as enp,
            tc.tile_pool(name="psA", bufs=4, space="PSUM") as psA,
            tc.tile_pool(name="psS", bufs=2, space="PSUM") as psS,
            tc.tile_pool(name="psM", bufs=1, space="PSUM") as psM,
        ):
            # ---------------- DMA emission (one queue, priority order) ----
            # tiny consts first (they gate the negM recipe and masks), then
            # whTs+hidf (the hid-bias matmuls cover the weTs/encTf[0] DMA
            # window on the PE), then the pass-1 stream, then pass-2 chunks.
            vcol_sb = consts.tile([128, DC], f32)
            nc.sync.dma_start(out=vcol_sb, in_=vcol_d[:, :])
            bcol_sb = consts.tile([128, DC], f32)
            nc.sync.dma_start(out=bcol_sb, in_=bcol_d[:, :])
            posf_sb = consts.tile([128, nf], f32)
            nc.sync.dma_start(out=posf_sb, in_=posf_d[:, :])
            lenmap_sb = consts.tile([128, nf], f32)
            nc.sync.dma_start(out=lenmap_sb, in_=lenmap_d[:, :])
            own0_sb = consts.tile([128, nf], f32)
            nc.sync.dma_start(out=own0_sb, in_=own0_d[:, :])
            whTs_sb = consts.tile([128, DC, d], bf16)
            nc.sync.dma_start(out=whTs_sb, in_=whTs_d[:, :, :])
            hidf_sb = consts.tile([128, DC, nt], bf16)
            nc.sync.dma_start(out=hidf_sb, in_=hidf_d[:, :, :])
            weTs_sb = consts.tile([128, DC, d], bf16)
            nc.sync.dma_start(out=weTs_sb, in_=weTs_d[:, :, :])
            et_tiles = []
            for t_ in range(nt):
                et = etp.tile([128, DC, ST], bf16, tag="et")
                nc.sync.dma_start(out=et, in_=encTf_d[t_])
                et_tiles.append(et)
            en2_tiles = []
            for f in range(nf):
                en2 = enf.tile([128, d], bf16, tag="en2")
                nc.sync.dma_start(out=en2, in_=encf_d[f])
                en2_tiles.append(en2)

            # ---------------- small constants ----------------------------
            ones_sb = consts.tile([128, 1], f32)
            nc.vector.memset(ones_sb, 1.0)
            ones_row = consts.tile([1, 128], f32)
            nc.vector.memset(ones_row, 1.0)
            # Upper bound M = sum|v| >= any score (|tanh|<=1): replaces the
            # serial max-reduce in the softmax.
            vabs = consts.tile([128, 1], f32)
            nc.vector.reduce_sum(
                out=vabs, in_=vcol_sb, axis=mybir.AxisListType.X,
                apply_absolute_value=True,
            )
            psv = psS.tile([1, 1], f32, tag="s", name="psv")
            nc.tensor.matmul(psv, ones_sb[:, 0:1], vabs, start=True, stop=True)
            mtot = consts.tile([1, 1], f32)
            nc.vector.tensor_copy(mtot, psv)
            psb = psS.tile([128, 1], f32, tag="s", name="psb")
            nc.tensor.matmul(psb, ones_row[:, :], mtot[:, :], start=True, stop=True)
            negM = consts.tile([128, 1], f32)
            nc.scalar.mul(negM, psb, -1.0)

            # masks from host-relayout index tensors: valid = pos < len,
            # then split by batch-slot ownership.
            valid_sb = consts.tile([128, nf], f32)
            nc.vector.scalar_tensor_tensor(
                valid_sb, posf_sb, 1.0, lenmap_sb, op0=Alu.mult, op1=Alu.is_lt
            )
            mask0 = consts.tile([128, nf], f32)
            nc.vector.scalar_tensor_tensor(
                mask0, valid_sb, 1.0, own0_sb, op0=Alu.mult, op1=Alu.mult
            )
            mask1 = consts.tile([128, nf], f32)
            nc.vector.scalar_tensor_tensor(
                mask1, valid_sb, 1.0, mask0, op0=Alu.mult, op1=Alu.subtract
            )

            # ---------------- hid bias: (hid @ Wh^T + b)^T ----------------
            # stationary Wh^T chunk [128d, 128e] x moving hidf [128d, nt]
            # -> [128e, nt] directly in the layout the tanh bias needs.
            bias_all = consts.tile([128, DC, nt], f32)
            for ec in range(DC):
                psh = psS.tile([128, nt], f32, tag="s", name=f"psh{ec}")
                for kc in range(DC):
                    nc.tensor.matmul(
                        psh,
                        whTs_sb[:, kc, ec * 128:(ec + 1) * 128],
                        hidf_sb[:, kc, :],
                        start=(kc == 0),
                        stop=(kc == DC - 1),
                    )
                nc.scalar.activation(
                    bias_all[:, ec, :], psh, Identity, bias=bcol_sb[:, ec:ec + 1]
                )

            # ---------------- pass 1 + pipelined softmax / pass 2 ---------
            scores_sb = consts.tile([128, nf], f32)
            exp_sb = consts.tile([128, nf], f32)
            attn2b = consts.tile([128, nf, 2], bf16)
            mexp0 = consts.tile([128, nf], f32)
            mexp1 = consts.tile([128, nf], f32)
            psums01 = consts.tile([128, 2], f32)
            cps = psM.tile([BL, d], f32, tag="m", name="cps")

            pend = None            # (tiles, accs) of the previous group
            p2_emitted = 0         # flat chunks whose pass-2 mm is emitted

            def emit_reduces(tiles, accs):
                # partition-reduce each acc column block into scores_sb.
                # All chunks of the pending group go into one PSUM tile
                # (separate cols) so nothing serializes on ring reuse.
                sps = psS.tile([128, NPT * len(tiles)], f32, tag="s")
                for j, t_ in enumerate(tiles):
                    for c in range(NPT):
                        nc.tensor.matmul(
                            sps[:, j * NPT + c:j * NPT + c + 1],
                            accs[t_][:, c * 128:(c + 1) * 128],
                            ones_sb[:, 0:1],
                            start=True,
                            stop=True,
                        )
                f0 = tiles[0] * NPT
                f1 = tiles[-1] * NPT + NPT
                nc.vector.tensor_copy(scores_sb[:, f0:f1], sps)

            def emit_softmax(tiles):
                f0 = tiles[0] * NPT
                f1 = tiles[-1] * NPT + NPT
                nc.scalar.activation(
                    exp_sb[:, f0:f1], scores_sb[:, f0:f1], Exp, bias=negM[:, 0:1]
                )
                nc.vector.scalar_tensor_tensor(
                    attn2b[:, f0:f1, 0], exp_sb[:, f0:f1], 1.0, mask0[:, f0:f1],
                    op0=Alu.mult, op1=Alu.mult,
                )
                nc.vector.scalar_tensor_tensor(
                    attn2b[:, f0:f1, 1], exp_sb[:, f0:f1], 1.0, mask1[:, f0:f1],
                    op0=Alu.mult, op1=Alu.mult,
                )

            def emit_pass2(tiles):
                nonlocal p2_emitted
                f0 = tiles[0] * NPT
                f1 = tiles[-1] * NPT + NPT
                for f in range(f0, f1):
                    for h in range(2):
                        nc.tensor.matmul(
                            cps[:, h * 512:(h + 1) * 512],
                            attn2b[:, f, :],
                            en2_tiles[f][:, h * 512:(h + 1) * 512],
                            start=(f == 0),
                            stop=(f == nf - 1),
                        )
                p2_emitted = f1

            for tiles in groups:
                accs = {}
                for ec in range(DC):
                    pss = {
                        t_: psA.tile([128, ST], f32, tag="proj", name=f"ps{t_}_{ec}")
                        for t_ in tiles
                    }
                    for kc in range(DC):
                        for t_ in tiles:
                            nc.tensor.matmul(
                                pss[t_],
                                weTs_sb[:, kc, ec * 128:(ec + 1) * 128],
                                et_tiles[t_][:, kc, :],
                                start=(kc == 0),
                                stop=(kc == DC - 1),
                            )
                    # deferred post-work of the previous group, staged so the
                    # PE queue always has matmul runway ahead of the deps.
                    if pend is not None:
                        if ec == 1:
                            emit_reduces(*pend)
                        elif ec == 3:
                            emit_softmax(pend[0])
                        elif ec == 5:
                            emit_pass2(pend[0])
                            pend = None
                    for t_ in tiles:
                        en = enp.tile([128, ST], f32, tag="en")
                        nc.scalar.activation(
                            en, pss[t_], Tanh, bias=bias_all[:, ec, t_:t_ + 1]
                        )
                        if ec == 0:
                            acc = enp.tile([128, ST], f32, tag="acc", bufs=5)
                            accs[t_] = acc
                            nc.vector.tensor_scalar_mul(acc, en, vcol_sb[:, 0:1])
                        else:
                            nc.vector.scalar_tensor_tensor(
                                accs[t_], en, vcol_sb[:, ec:ec + 1], accs[t_],
                                op0=Alu.mult, op1=Alu.add,
                            )
                pend = (tiles, accs)

            # tail: post-work of the last group
            emit_reduces(*pend)
            emit_softmax(pend[0])
            if stage == "p1":
                nc.gpsimd.dma_start(out=out_d[:, :], in_=scores_sb)
            elif stage == "sm":
                nc.gpsimd.dma_start(out=out_d[:, :], in_=exp_sb)
            else:
                # softmax denominators -> 1/sum, BEFORE the last pass-2 mms
                # so only the final scale remains on the tail chain.
                nc.vector.scalar_tensor_tensor(
                    mexp0, exp_sb, 1.0, mask0, op0=Alu.mult, op1=Alu.mult,
                    accum_out=psums01[:, 0:1],
                )
                nc.vector.scalar_tensor_tensor(
                    mexp1, exp_sb, 1.0, mask1, op0=Alu.mult, op1=Alu.mult,
                    accum_out=psums01[:, 1:2],
                )
                pst = psS.tile([BL, 1], f32, tag="s", name="pst")
                nc.tensor.matmul(pst, psums01, ones_sb[:, 0:1], start=True, stop=True)
                rinv2 = consts.tile([BL, 1], f32)
                nc.vector.reciprocal(rinv2, pst)
                emit_pass2(pend[0])
                assert p2_emitted == nf
                ctx_sb = consts.tile([BL, d], f32)
                nc.vector.tensor_scalar_mul(ctx_sb, cps, rinv2)
                nc.gpsimd.dma_start(out=out_d[:, :], in_=ctx_sb)

    nc.compile()
    return nc


def _get_nc(nt, stage="all"):
    key = (nt, stage)
    if key not in _NC_CACHE:
        _NC_CACHE[key] = _build_program(nt, stage)
    return _NC_CACHE[key]


def _plan(lengths):
    """Pair batches (longest padded length with shortest) so every core's
    two batches need the same, minimal number of 512-wide s-tiles."""
    l = np.asarray(lengths, dtype=np.int64)
    c = (np.clip(l, 1, S) + ST - 1) // ST          # tiles per batch, >= 1
    order = np.argsort(-c, kind="stable")
    pairs = [(int(order[i]), int(order[B - 1 - i])) for i in range(NCORES)]
    nt = int(max(c[a] + c[b] for a, b in pairs))
    return pairs, c, nt


def _make_in_maps(encoder_outputs, hidden, lengths, W, b, v):
    import ml_dtypes

    bf16 = ml_dtypes.bfloat16
    enc = np.asarray(encoder_outputs, dtype=np.float32)
    hid = np.asarray(hidden, dtype=np.float32)
    len_ = np.asarray(lengths, dtype=np.int64)
    Wn = np.asarray(W, dtype=np.float32)
    bn = np.asarray(b, dtype=np.float32)
    vn = np.asarray(v, dtype=np.float32)

    pairs, c, nt = _plan(len_)
    nf = nt * NPT

    # partition-major: [p, c, e] = W.T[c*128 + p, e] so each partition's DMA
    # line is one contiguous block.
    weTs = np.ascontiguousarray(
        Wn[:, D:].T.reshape(DC, 128, D).transpose(1, 0, 2).astype(bf16)
    )
    whTs = np.ascontiguousarray(
        Wn[:, :D].T.reshape(DC, 128, D).transpose(1, 0, 2).astype(bf16)
    )
    bcol = np.ascontiguousarray(bn.reshape(DC, 128).T)
    vcol = np.ascontiguousarray(vn.reshape(DC, 128).T)

    in_maps = []
    for a, b_ in pairs:
        na, nb = int(c[a]), int(c[b_])
        packed = np.zeros((nt * ST, D), dtype=np.float32)
        packed[:na * ST] = enc[a, :na * ST]
        packed[na * ST:(na + nb) * ST] = enc[b_, :nb * ST]
        packed = packed.astype(bf16)
        encTf = np.ascontiguousarray(
            packed.reshape(nt, ST, DC, 128).transpose(0, 3, 2, 1)
        )
        encf = packed.reshape(nf, 128, D)

        hidf = np.zeros((D, nt), dtype=np.float32)
        hidf[:, :na] = hid[a][:, None]
        hidf[:, na:na + nb] = hid[b_][:, None]
        hidf = np.ascontiguousarray(
            hidf.astype(bf16).reshape(DC, 128, nt).transpose(1, 0, 2)
        )

        posf = np.full((128, nf), 1.0e9, dtype=np.float32)
        lenmap = np.zeros((128, nf), dtype=np.float32)
        own0 = np.zeros((128, nf), dtype=np.float32)
        p = np.arange(128, dtype=np.float32)
        for f in range(nf):
            t = f // NPT
            if t < na:
                posf[:, f] = f * 128 + p
                lenmap[:, f] = float(len_[a])
                own0[:, f] = 1.0
            elif t < na + nb:
                posf[:, f] = (f - na * NPT) * 128 + p
                lenmap[:, f] = float(len_[b_])

        in_maps.append(
            dict(
                encTf=encTf, encf=np.ascontiguousarray(encf),
                weTs=weTs, whTs=whTs, hidf=hidf,
                bcol=bcol, vcol=vcol,
                posf=posf, lenmap=lenmap, own0=own0,
            )
        )
    return in_maps, pairs, nt


def run(inputs, trace=False, stage="all"):
    """Run on 8 NeuronCores; returns (output [B,1,D], BassKernelResults)."""
    from concourse.bass_utils import run_bass_kernel_spmd

    in_maps, pairs, nt = _make_in_maps(**inputs)
    nc = _get_nc(nt, stage)
    r = run_bass_kernel_spmd(
        nc, in_maps, core_ids=list(range(NCORES)), trace=trace
    )
    if stage != "all":
        out = np.stack(
            [np.asarray(r.results[i]["ctx_out"]) for i in range(NCORES)], axis=0
        )
        return out, r, pairs
    out = np.empty((B, 1, D), dtype=np.float32)
    for i, (a, b_) in enumerate(pairs):
        ctx = np.asarray(r.results[i]["ctx_out"])
        out[a, 0] = ctx[0]
        out[b_, 0] = ctx[1]
    return out, r


def kernel(encoder_outputs, hidden, lengths, W, b, v):
    out, _ = run(
        dict(
            encoder_outputs=encoder_outputs,
            hidden=hidden,
            lengths=lengths,
            W=W,
            b=b,
            v=v,
        )
    )
    return out
